# revision 3
# baseline (speedup 1.0000x reference)
"""GQA attention (B=2, S=2048, D=2048, 32 q heads / 8 kv heads, RoPE, causal)
sharded over 8 trn2 NeuronCores: tensor-parallel over heads (4 q heads + 1 kv
head per core), AllToAll to redistribute attention output by sequence slice,
each core computes its 512-row slice of the output projection.

Self-contained: hardcodes shapes; builds one SPMD Bass/Tile program and runs it
via run_bass_kernel_spmd on cores 0-7.
"""

import os
import sys

import numpy as np

for _p in ("/opt/trn_rl_repo", "/root/.axon_site/_ro/trn_rl_repo"):
    if os.path.isdir(_p) and _p not in sys.path:
        sys.path.insert(0, _p)

B = 2
S = 2048
D = 2048
HD = 64          # head dim
HQ = 4           # q heads per core
NCORES = 8
ROPE_THETA = 10000.0
P = 128

_CACHE: dict = {}


def _build_program():
    import concourse.bacc as bacc
    import concourse.tile as tile
    from concourse import mybir
    from concourse.masks import make_identity

    dt = mybir.dt.float32
    AF = mybir.ActivationFunctionType
    OP = mybir.AluOpType

    nc = bacc.Bacc("TRN2", target_bir_lowering=False, debug=False, num_devices=NCORES)

    x = nc.dram_tensor("x", [B, S, D], dt, kind="ExternalInput")
    wqT = nc.dram_tensor("wqT", [D, 256], dt, kind="ExternalInput")
    wk2T = nc.dram_tensor("wk2T", [D, 128], dt, kind="ExternalInput")
    wvT = nc.dram_tensor("wvT", [D, 64], dt, kind="ExternalInput")
    woT = nc.dram_tensor("woT", [D, D], dt, kind="ExternalInput")
    ctab = nc.dram_tensor("ctab", [P, S], dt, kind="ExternalInput")
    stab = nc.dram_tensor("stab", [P, S], dt, kind="ExternalInput")
    pswT = nc.dram_tensor("pswT", [P, P], dt, kind="ExternalInput")
    out = nc.dram_tensor("out", [512, D], dt, kind="ExternalOutput")
    a2a_in = nc.dram_tensor("a2a_in", [2048, 512], dt)
    a2a_out = nc.dram_tensor("a2a_out", [2048, 512], dt)

    with tile.TileContext(nc) as tc:
        with tc.tile_pool(name="singles", bufs=1) as singles:
            ident = singles.tile([P, P], dt)
            make_identity(nc, ident)
            ones64 = singles.tile([P, HD], dt)
            nc.vector.memset(ones64, 1.0)
            c_sb = singles.tile([P, S], dt)
            nc.sync.dma_start(c_sb, ctab.ap())
            s_sb = singles.tile([P, S], dt)
            nc.sync.dma_start(s_sb, stab.ap())
            psw_sb = singles.tile([P, P], dt)
            nc.sync.dma_start(psw_sb, pswT.ap())
            wq_sb = singles.tile([P, 16, 256], dt)
            nc.sync.dma_start(wq_sb, wqT.ap().rearrange("(ko p) m -> p ko m", p=P))
            wk_sb = singles.tile([P, 16, 128], dt)
            nc.sync.dma_start(wk_sb, wk2T.ap().rearrange("(ko p) m -> p ko m", p=P))
            wv_sb = singles.tile([P, 16, 64], dt)
            nc.sync.dma_start(wv_sb, wvT.ap().rearrange("(ko p) m -> p ko m", p=P))

            for b in range(B):
                with tc.tile_pool(name=f"qkv{b}", bufs=1) as qkv:
                    qT = qkv.tile([P, 2, S], dt)      # 2 head-pairs stacked [A(0:64)|B(64:128)]
                    k2T = qkv.tile([P, S], dt)        # kv head duplicated in both halves
                    vsb = qkv.tile([P, 16, HD], dt)   # V chunks: [s-part, kc, d]

                    # ---- x^T production + QKV projection, per 512-col s-block ----
                    with tc.tile_pool(name=f"xt{b}", bufs=2) as xtp, \
                         tc.tile_pool(name=f"xnat{b}", bufs=3) as xnp, \
                         tc.tile_pool(name=f"tmp{b}", bufs=4) as tmp, \
                         tc.tile_pool(name=f"pst{b}", bufs=2, space="PSUM") as pst, \
                         tc.tile_pool(name=f"psq{b}", bufs=2, space="PSUM") as psq, \
                         tc.tile_pool(name=f"psw{b}", bufs=2, space="PSUM") as psw:
                        for n4 in range(4):
                            xTblk = xtp.tile([P, 16, 512], dt, tag="xT")
                            for sc4 in range(4):
                                sc = n4 * 4 + sc4
                                xa = xnp.tile([P, D], dt, tag="xa")
                                nc.sync.dma_start(xa, x.ap()[b, sc * P:(sc + 1) * P, :])
                                for dg in range(4):
                                    pt = pst.tile([P, 512], dt, tag="pt")
                                    for j in range(4):
                                        dc = dg * 4 + j
                                        nc.tensor.transpose(
                                            pt[:, j * P:(j + 1) * P],
                                            xa[:, dc * P:(dc + 1) * P],
                                            ident,
                                        )
                                    dest = xTblk[:, dg * 4:dg * 4 + 4, sc4 * P:(sc4 + 1) * P]
                                    nc.vector.tensor_copy(
                                        dest, pt[:, :].rearrange("p (a c) -> p a c", a=4)
                                    )
                            # Q (pairs m=0,1) and K2 (m=2) projections for this s-block
                            for m in range(3):
                                ps = psq.tile([P, 512], dt, tag="ps")
                                for k in range(16):
                                    lhsT = wq_sb[:, k, m * P:(m + 1) * P] if m < 2 else wk_sb[:, k, :]
                                    nc.tensor.matmul(
                                        ps, lhsT, xTblk[:, k, :],
                                        start=(k == 0), stop=(k == 15),
                                    )
                                raw = tmp.tile([P, 512], dt, tag="raw")
                                nc.scalar.copy(out=raw, in_=ps)
                                sw = psw.tile([P, 512], dt, tag="sw")
                                nc.tensor.matmul(sw, psw_sb, raw, start=True, stop=True)
                                u = tmp.tile([P, 512], dt, tag="u")
                                nc.vector.tensor_tensor(
                                    u, sw, s_sb[:, n4 * 512:(n4 + 1) * 512], OP.mult)
                                t2 = tmp.tile([P, 512], dt, tag="t2")
                                nc.vector.tensor_tensor(
                                    t2, raw, c_sb[:, n4 * 512:(n4 + 1) * 512], OP.mult)
                                dest = qT[:, m, n4 * 512:(n4 + 1) * 512] if m < 2 \
                                    else k2T[:, n4 * 512:(n4 + 1) * 512]
                                nc.vector.tensor_tensor(dest, u, t2, OP.add)
                            # V for the 4 s-chunks of this block
                            pv = psq.tile([P, 256], dt, tag="pv")
                            for j in range(4):
                                sc = n4 * 4 + j
                                for k in range(16):
                                    nc.tensor.matmul(
                                        pv[:, j * HD:(j + 1) * HD],
                                        xTblk[:, k, j * P:(j + 1) * P],
                                        wv_sb[:, k, :],
                                        start=(k == 0), stop=(k == 15),
                                    )
                            nc.scalar.copy(
                                out=vsb[:, n4 * 4:n4 * 4 + 4, :],
                                in_=pv[:, :].rearrange("p (a c) -> p a c", a=4),
                            )

                    # ---- attention, orientation B (scores^T [k, q]) ----
                    for p in range(2):  # head pair
                        with tc.tile_pool(name=f"at{b}{p}", bufs=2) as atp, \
                             tc.tile_pool(name=f"ps_s{b}{p}", bufs=1, space="PSUM") as pss, \
                             tc.tile_pool(name=f"ps_pv{b}{p}", bufs=2, space="PSUM") as pspv, \
                             tc.tile_pool(name=f"ps_on{b}{p}", bufs=2, space="PSUM") as pson:
                            for qh in range(2):
                                qcs = [2 * qh, 2 * qh + 1]
                                pvps = {qc: pspv.tile([P, 512], dt, name="pvacc", tag="pv") for qc in qcs}
                                onps = {qc: pson.tile([P, 512], dt, name="onacc", tag="on") for qc in qcs}
                                for kc in range(4 * qcs[-1] + 4):
                                    act_qcs = [qc for qc in qcs if kc < 4 * (qc + 1)]
                                    w = 512 * len(act_qcs)
                                    sA = pss.tile([P, 1024], dt, tag="sA")
                                    sB = pss.tile([P, 1024], dt, tag="sB")
                                    for j, qc in enumerate(act_qcs):
                                        for h in range(2):
                                            ps = sA if h == 0 else sB
                                            nc.tensor.matmul(
                                                ps[:, j * 512:(j + 1) * 512],
                                                k2T[64 * h:64 * (h + 1), kc * P:(kc + 1) * P],
                                                qT[64 * h:64 * (h + 1), p, qc * 512:(qc + 1) * 512],
                                                start=True, stop=True,
                                                tile_position=(64 * h, 0),
                                                skip_group_check=True,
                                            )
                                    pA = atp.tile([P, 1024], dt, tag="pA")
                                    pB = atp.tile([P, 1024], dt, tag="pB")
                                    nc.scalar.activation(pA[:, :w], sA[:, :w], AF.Exp)
                                    nc.scalar.activation(pB[:, :w], sB[:, :w], AF.Exp)
                                    for j, qc in enumerate(act_qcs):
                                        if kc >= 4 * qc:  # diagonal chunk: causal staircase
                                            for pt_ in (pA, pB):
                                                nc.gpsimd.affine_select(
                                                    out=pt_[:, j * 512:(j + 1) * 512],
                                                    in_=pt_[:, j * 512:(j + 1) * 512],
                                                    pattern=[[1, 512]],
                                                    compare_op=OP.is_ge,
                                                    fill=0.0,
                                                    base=512 * qc - P * kc,
                                                    channel_multiplier=-1,
                                                )
                                    for j, qc in enumerate(act_qcs):
                                        first = kc == 0
                                        last = kc == 4 * (qc + 1) - 1
                                        nc.tensor.matmul(
                                            pvps[qc][0:64], vsb[:, kc, :], pA[:, j * 512:(j + 1) * 512],
                                            start=first, stop=last, tile_position=(0, 0),
                                            skip_group_check=True)
                                        nc.tensor.matmul(
                                            pvps[qc][64:128], vsb[:, kc, :], pB[:, j * 512:(j + 1) * 512],
                                            start=first, stop=last, tile_position=(0, 64),
                                            skip_group_check=True)
                                        nc.tensor.matmul(
                                            onps[qc][0:64], ones64, pA[:, j * 512:(j + 1) * 512],
                                            start=first, stop=last, tile_position=(0, 0),
                                            skip_group_check=True)
                                        nc.tensor.matmul(
                                            onps[qc][64:128], ones64, pB[:, j * 512:(j + 1) * 512],
                                            start=first, stop=last, tile_position=(0, 64),
                                            skip_group_check=True)
                                for qc in qcs:
                                    ln = atp.tile([P, 512], dt, tag="ln")
                                    nc.scalar.activation(ln, onps[qc], AF.Ln)
                                    rc = atp.tile([P, 512], dt, tag="rc")
                                    nc.scalar.activation(rc, ln, AF.Exp, scale=-1.0)
                                    at = atp.tile([P, 512], dt, tag="at")
                                    nc.vector.tensor_tensor(at, pvps[qc], rc, OP.mult)
                                    r_dest = 4 * b + qc
                                    nc.sync.dma_start(
                                        a2a_in.ap()[256 * r_dest + P * p:256 * r_dest + P * (p + 1), :],
                                        at,
                                    )

            # ---- AllToAll: redistribute attn^T by sequence slice ----
            nc.gpsimd.collective_compute(
                "AllToAll",
                mybir.AluOpType.bypass,
                replica_groups=[list(range(NCORES))],
                ins=[a2a_in.ap().opt()],
                outs=[a2a_out.ap().opt()],
            )

            # ---- output projection for this core's 512 rows ----
            with tc.tile_pool(name="gp", bufs=1) as gp, \
                 tc.tile_pool(name="wop", bufs=2) as wop, \
                 tc.tile_pool(name="otmp", bufs=3) as otmp, \
                 tc.tile_pool(name="pso", bufs=2, space="PSUM") as pso:
                g_sb = gp.tile([P, 16, 512], dt)
                nc.sync.dma_start(g_sb, a2a_out.ap().rearrange("(ko p) q -> p ko q", p=P))
                for n in range(4):
                    wo_sb = wop.tile([P, 16, 512], dt, tag="wo")
                    nc.sync.dma_start(
                        wo_sb, woT.ap()[:, n * 512:(n + 1) * 512].rearrange("(ko p) f -> p ko f", p=P))
                    for m in range(4):
                        po = pso.tile([P, 512], dt, tag="po")
                        for k in range(16):
                            nc.tensor.matmul(
                                po, g_sb[:, k, m * P:(m + 1) * P], wo_sb[:, k, :],
                                start=(k == 0), stop=(k == 15))
                        ob = otmp.tile([P, 512], dt, tag="ob")
                        nc.scalar.copy(out=ob, in_=po)
                        nc.sync.dma_start(out.ap()[m * P:(m + 1) * P, n * 512:(n + 1) * 512], ob)

    nc.compile()
    return nc


def _host_prep(Wq, Wk, Wv, Wo):
    """Per-core weight slices (head-dim permuted, transposed) + rope tables."""
    perm = np.concatenate([np.arange(0, HD, 2), np.arange(1, HD, 2)])
    per_core = []
    for r in range(NCORES):
        wq = Wq[256 * r:256 * (r + 1)].reshape(HQ, HD, D)[:, perm].reshape(256, D)
        wqT = np.ascontiguousarray(wq.T) * np.float32(0.125)
        wk = Wk[HD * r:HD * (r + 1)][perm]
        wk2T = np.ascontiguousarray(np.concatenate([wk, wk], 0).T)
        wvT = np.ascontiguousarray(Wv[HD * r:HD * (r + 1)].T)
        per_core.append((wqT, wk2T, wvT))
    woT = np.ascontiguousarray(Wo.T)

    half = HD // 2
    inv = 1.0 / (ROPE_THETA ** (np.arange(half, dtype=np.float64) * 2.0 / HD))
    ang = np.arange(S, dtype=np.float64)[None, :] * inv[:, None]  # [32, S]
    ctab = np.ascontiguousarray(np.tile(np.cos(ang), (4, 1)).astype(np.float32))
    stab = np.ascontiguousarray(np.tile(np.sin(ang), (4, 1)).astype(np.float32))

    I32 = np.eye(32, dtype=np.float32)
    z = np.zeros((32, 32), np.float32)
    blk = np.block([[z, -I32], [I32, z]])
    pswT = np.ascontiguousarray(np.kron(np.eye(2, dtype=np.float32), blk).T)
    return per_core, woT, ctab, stab, pswT


def _get_nc():
    if "nc" not in _CACHE:
        _CACHE["nc"] = _build_program()
    return _CACHE["nc"]


def make_in_maps(x, Wq, Wk, Wv, Wo):
    x = np.ascontiguousarray(np.asarray(x, np.float32))
    Wq = np.asarray(Wq, np.float32)
    Wk = np.asarray(Wk, np.float32)
    Wv = np.asarray(Wv, np.float32)
    Wo = np.asarray(Wo, np.float32)
    per_core, woT, ctab, stab, pswT = _host_prep(Wq, Wk, Wv, Wo)
    in_maps = []
    for r in range(NCORES):
        wqT, wk2T, wvT = per_core[r]
        in_maps.append({
            "x": x, "wqT": wqT, "wk2T": wk2T, "wvT": wvT, "woT": woT,
            "ctab": ctab, "stab": stab, "pswT": pswT,
        })
    return in_maps


def run(x, Wq, Wk, Wv, Wo, trace=False):
    from concourse.bass_utils import run_bass_kernel_spmd

    nc = _get_nc()
    in_maps = make_in_maps(x, Wq, Wk, Wv, Wo)
    res = run_bass_kernel_spmd(nc, in_maps, list(range(NCORES)), trace=trace)
    out = np.concatenate([res.results[r]["out"] for r in range(NCORES)], axis=0)
    return out.reshape(B, S, D), res


def kernel(x, Wq, Wk, Wv, Wo):
    out, _ = run(x, Wq, Wk, Wv, Wo)
    return out
